# revision 7
# baseline (speedup 1.0000x reference)
"""EFLSTM Trainium2 kernel: 8-core tensor-parallel LSTM + fused head.

Strategy (8 NeuronCores, SPMD single program, per-core data differs):
  - Tensor-parallel over the 4H gate dimension: core k owns gate rows
    [k*128:(k+1)*128] of each of the four gates (i, f, g, o), so it computes
    h rows [k*128:(k+1)*128] each timestep.  A per-step AllGather rebuilds
    the full transposed hidden state on every core.
  - TWO INTERLEAVED HALF-BATCH CHAINS: the batch is split into two
    independent 64-row LSTMs.  Their per-step AllGathers alternate on the
    collective engine, which pipelines them (~4.2us/op measured vs ~10us
    for a single dependent chain), hiding collective latency behind the
    other half's compute.
  - Input projection x @ W_ih.T is folded into the recurrent PSUM
    accumulation (x pre-transposed on host with a ones-row folding the
    bias into the matmul).  Projection matmuls are independent of h and
    fill the AllGather latency window.
  - DMA-lean: xT is partition-major, loaded 8 steps per DMA (both halves
    in one load); head outputs staged in SBUF, stored 64 steps per DMA.
  - Matmul operands and the h exchange are bf16 (fp32 PSUM, fp32 cell
    state); gate nonlinearities split per gate chunk so the c-update
    overlaps the o-gate matmuls.
  - FC head fused per-step per half (one 128-wide slice of C per core;
    cores k and k+4 duplicate a slice, fc2 weights pre-scaled by 0.5);
    partials accumulate token-major; one ReduceScatter + log_softmax at
    the end.
"""

import numpy as np
import ml_dtypes

import concourse.bacc as bacc
import concourse.mybir as mybir
import concourse.tile as tile
from concourse.bass_utils import run_bass_kernel_spmd

F32 = mybir.dt.float32
BF16 = mybir.dt.bfloat16
AF = mybir.ActivationFunctionType
ALU = mybir.AluOpType

N_CORES = 8
B = 128
BH = 64                   # half-batch (one chain)
T = 512
DIMS = (300, 74, 35)
D = sum(DIMS)  # 409
DP = D + 1     # 410: extra ones-row folds the gate bias into the matmul
H = 1024
G = 4 * H
C = 512
O = 7
HSL = H // N_CORES        # 128 h rows per core
GSL = 4 * HSL             # 512 gate rows per core
KC_X = [128, 128, 128, DP - 3 * 128]   # contraction chunks over D+1
NKX = len(KC_X)
NKH = H // 128            # 8 contraction chunks over H
TOK = B * T
XB = 8                    # timesteps per xT load
ZB = 64                   # timesteps per output store


def build_kernel(t_steps=T):
    nc = bacc.Bacc("TRN2", target_bir_lowering=False, debug=False,
                   num_devices=N_CORES)

    xb = min(XB, t_steps)
    zb = min(ZB, t_steps)
    assert t_steps % xb == 0 and t_steps % zb == 0

    # xT partition-major: [T/XB, 128, XB, NKX, B] so one DMA loads XB steps
    xT = nc.dram_tensor("xT", [t_steps // xb, 128, xb, NKX, B], BF16,
                        kind="ExternalInput")
    wihT = nc.dram_tensor("wihT", [NKX, 128, GSL], BF16, kind="ExternalInput")
    whhT = nc.dram_tensor("whhT", [NKH, 128, GSL], BF16, kind="ExternalInput")
    fc1wT = nc.dram_tensor("fc1wT", [NKH, 128, 128], BF16, kind="ExternalInput")
    fc1b = nc.dram_tensor("fc1b", [128, 1], F32, kind="ExternalInput")
    fc2wT = nc.dram_tensor("fc2wT", [128, O], BF16, kind="ExternalInput")
    fc2b = nc.dram_tensor("fc2b", [128, O], F32, kind="ExternalInput")

    n_tok_loc = B * t_steps // N_CORES
    out_sh = nc.dram_tensor("out_sh", [n_tok_loc, O], F32, kind="ExternalOutput")

    with tile.TileContext(nc) as tc:
        with (
            tc.tile_pool(name="const", bufs=1) as const,
            tc.tile_pool(name="xtp", bufs=2) as xtp,
            tc.tile_pool(name="pg", bufs=4, space="PSUM") as pgp,
            tc.tile_pool(name="ph", bufs=2, space="PSUM") as php,
            tc.tile_pool(name="pf", bufs=2, space="PSUM") as pfp,
            tc.tile_pool(name="ew", bufs=4) as ewp,
            tc.tile_pool(name="zst", bufs=4) as zstp,
            tc.tile_pool(name="htp", bufs=6) as htp,
            tc.tile_pool(name="dram", bufs=4, space="DRAM") as dramp,
            tc.tile_pool(name="dram1", bufs=1, space="DRAM") as dramp1,
        ):
            # ---- resident weights ----
            wih_sb = const.tile([128, NKX, GSL], BF16)
            nc.sync.dma_start(wih_sb[:], wihT[:].rearrange("k p g -> p k g"))
            whh_sb = const.tile([128, NKH, GSL], BF16)
            nc.sync.dma_start(whh_sb[:], whhT[:].rearrange("k p g -> p k g"))
            fc1_sb = const.tile([128, NKH, 128], BF16)
            nc.sync.dma_start(fc1_sb[:], fc1wT[:].rearrange("k p c -> p k c"))
            fc1b_sb = const.tile([128, 1], F32)
            nc.sync.dma_start(fc1b_sb[:], fc1b[:])
            fc2w_sb = const.tile([128, O], BF16)
            nc.sync.dma_start(fc2w_sb[:], fc2wT[:])
            fc2b_sb = const.tile([128, O], F32)
            nc.sync.dma_start(fc2b_sb[:], fc2b[:])

            # persistent cell state slice per half
            c_sb = [const.tile([128, BH], F32, name=f"c_sb{i}")
                    for i in range(2)]

            out_part = dramp1.tile([n_tok_loc * N_CORES, O], F32)
            out_bt = out_part[:].rearrange("(b t) o -> b t o", t=t_steps)

            hT_prev = [None, None]
            xts = None
            zstage = [None, None]
            # gate layout in pg: [i | f | g | o] x BH columns each
            gsl = [(mc * BH, (mc + 1) * BH) for mc in range(4)]

            for t in range(t_steps):
                # -- batched xT load (XB steps per DMA, both halves) --
                if t % xb == 0:
                    xts = xtp.tile([128, xb, NKX, B], BF16)
                    nc.scalar.dma_start(xts[:], xT[t // xb])

                for hf in range(2):
                    b0 = hf * BH
                    # -- projection matmuls for (t, hf), independent of h --
                    pg = pgp.tile([128, 4 * BH], F32)
                    for mc in range(4):
                        lo, hi = gsl[mc]
                        glo = mc * 128
                        for kc in range(NKX):
                            kk = KC_X[kc]
                            nc.tensor.matmul(
                                pg[:, lo:hi],
                                wih_sb[:kk, kc, glo:glo + 128],
                                xts[:kk, t % xb, kc, b0:b0 + BH],
                                start=(kc == 0),
                                stop=(t == 0 and kc == NKX - 1),
                                skip_group_check=True,
                            )
                    # -- recurrent matmuls (consume gathered h of step t-1) --
                    if t > 0:
                        for mc in range(4):
                            lo, hi = gsl[mc]
                            glo = mc * 128
                            for kc in range(NKH):
                                nc.tensor.matmul(
                                    pg[:, lo:hi],
                                    whh_sb[:, kc, glo:glo + 128],
                                    hT_prev[hf][:, kc, :],
                                    start=False,
                                    stop=(kc == NKH - 1),
                                    skip_group_check=True,
                                )

                    # -- head for step t-1 (emitted after rec so PE
                    #    prioritizes rec); z staged, stored every ZB steps --
                    if t > 0:
                        if (t - 1) % zb == 0:
                            zstage[hf] = zstp.tile([BH, zb, O], F32, name=f"zstage{hf}")
                        emit_head(nc, php, pfp, ewp, fc1_sb, fc1b_sb,
                                  fc2w_sb, hT_prev[hf], zstage[hf],
                                  (t - 1) % zb)
                        if (t - 1) % zb == zb - 1:
                            t0 = t - 1 - (zb - 1)
                            nc.scalar.dma_start(
                                out_bt[b0:b0 + BH, t0:t0 + zb, :], zstage[hf][:])

                    # -- elementwise: gates -> h slice (i,f,g,o chunk order;
                    #    c-update runs while the o-gate matmuls finish) --
                    gnl = ewp.tile([128, 4 * BH], F32)
                    nc.scalar.activation(gnl[:, 0:2 * BH], pg[:, 0:2 * BH],
                                         AF.Sigmoid)
                    if t > 0:
                        fcs = ewp.tile([128, BH], F32)
                        nc.vector.tensor_mul(fcs[:], gnl[:, BH:2 * BH],
                                             c_sb[hf][:])
                    nc.scalar.activation(gnl[:, 2 * BH:3 * BH],
                                         pg[:, 2 * BH:3 * BH], AF.Tanh)
                    ig = ewp.tile([128, BH], F32)
                    nc.vector.tensor_mul(ig[:], gnl[:, 0:BH],
                                         gnl[:, 2 * BH:3 * BH])
                    if t == 0:
                        nc.vector.tensor_copy(c_sb[hf][:], ig[:])
                    else:
                        nc.vector.tensor_add(c_sb[hf][:], fcs[:], ig[:])
                    tc_t = ewp.tile([128, BH], F32)
                    nc.scalar.activation(tc_t[:], c_sb[hf][:], AF.Tanh)
                    nc.scalar.activation(gnl[:, 3 * BH:4 * BH],
                                         pg[:, 3 * BH:4 * BH], AF.Sigmoid)
                    h_sl = ewp.tile([128, BH], BF16)
                    nc.vector.tensor_mul(h_sl[:], gnl[:, 3 * BH:4 * BH],
                                         tc_t[:])

                    # -- exchange: AllGather this half's h_T slice --
                    # per-chain DMA issue queues (A: sync, B: gpsimd) so one
                    # chain's gather-wait never blocks the other's issue
                    dma_eng = nc.sync if hf == 0 else nc.gpsimd
                    bnc_in = dramp.tile([128, BH], BF16)
                    dma_eng.dma_start(bnc_in[:], h_sl[:])
                    bnc_out = dramp.tile([H, BH], BF16, addr_space="Shared")
                    nc.gpsimd.collective_compute(
                        "AllGather",
                        ALU.bypass,
                        replica_groups=[list(range(N_CORES))],
                        ins=[bnc_in[:].opt()],
                        outs=[bnc_out[:].opt()],
                    )
                    hT = htp.tile([128, NKH, BH], BF16)
                    dma_eng.dma_start(
                        hT[:], bnc_out[:].rearrange("(k p) b -> p k b", p=128))
                    hT_prev[hf] = hT

            # head for the final step, per half
            for hf in range(2):
                b0 = hf * BH
                if (t_steps - 1) % zb == 0:
                    zstage[hf] = zstp.tile([BH, zb, O], F32, name=f"zstage{hf}")
                emit_head(nc, php, pfp, ewp, fc1_sb, fc1b_sb, fc2w_sb,
                          hT_prev[hf], zstage[hf], (t_steps - 1) % zb)
                t0 = t_steps - 1 - ((t_steps - 1) % zb)
                nc.sync.dma_start(out_bt[b0:b0 + BH, t0:t_steps, :],
                                  zstage[hf][:, 0:t_steps - t0, :])

            # ---- tail: ReduceScatter fc2 partials, bias + log_softmax ----
            rs_out = dramp1.tile([n_tok_loc, O], F32)
            nc.gpsimd.collective_compute(
                "ReduceScatter",
                ALU.add,
                replica_groups=[list(range(N_CORES))],
                ins=[out_part[:].opt()],
                outs=[rs_out[:].opt()],
            )
            n_chunks = n_tok_loc // 128
            for ch in range(n_chunks):
                z = ewp.tile([128, O], F32)
                nc.sync.dma_start(z[:], rs_out[ch * 128:(ch + 1) * 128, :])
                zb_ = ewp.tile([128, O], F32)
                nc.vector.tensor_add(zb_[:], z[:], fc2b_sb[:])
                mx = ewp.tile([128, 1], F32)
                nc.vector.reduce_max(mx[:], zb_[:], axis=mybir.AxisListType.X)
                sh = ewp.tile([128, O], F32)
                nc.vector.tensor_scalar_sub(sh[:], zb_[:], mx[:])
                ex = ewp.tile([128, O], F32)
                nc.scalar.activation(ex[:], sh[:], AF.Exp)
                sm = ewp.tile([128, 1], F32)
                nc.vector.reduce_sum(sm[:], ex[:], axis=mybir.AxisListType.X)
                lg = ewp.tile([128, 1], F32)
                nc.scalar.activation(lg[:], sm[:], AF.Ln)
                res = ewp.tile([128, O], F32)
                nc.vector.tensor_scalar_sub(res[:], sh[:], lg[:])
                nc.sync.dma_start(out_sh[ch * 128:(ch + 1) * 128, :], res[:])

    nc.compile()
    return nc


def emit_head(nc, php, pfp, ewp, fc1_sb, fc1b_sb, fc2w_sb, hT, zstage, zi):
    """fc1 (C slice) + relu + fc2 partial for one half-timestep."""
    ph = php.tile([128, BH], F32)
    for kc in range(NKH):
        nc.tensor.matmul(ph[:], fc1_sb[:, kc, :], hT[:, kc, :],
                         start=(kc == 0), stop=(kc == NKH - 1))
    hid = ewp.tile([128, BH], BF16)
    nc.scalar.activation(hid[:], ph[:], AF.Relu, bias=fc1b_sb[:])
    pf = pfp.tile([BH, O], F32)
    nc.tensor.matmul(pf[:], hid[:], fc2w_sb[:], start=True, stop=True)
    nc.vector.tensor_copy(zstage[:, zi, :], pf[:])


_CACHED = {}


def _get_kernel(t_steps):
    if t_steps not in _CACHED:
        _CACHED[t_steps] = build_kernel(t_steps)
    return _CACHED[t_steps]


def prep_inputs(m_text, m_audio, m_video, W_ih, W_hh, b_ih, b_hh,
                fc1_w, fc1_b, fc2_w, fc2_b, t_steps=T):
    """Host-side layout prep; returns per-core input maps."""
    bf = ml_dtypes.bfloat16
    x = np.concatenate([np.asarray(m_text), np.asarray(m_audio),
                        np.asarray(m_video)], axis=-1).astype(np.float32)
    b_, t_, d_ = x.shape
    assert (b_, d_) == (B, D) and t_ == t_steps
    xb = min(XB, t_steps)
    # xT partition-major: [T/xb, 128(p), xb, NKX(kc), B]
    xTf = np.zeros((t_steps, NKX * 128, B), np.float32)
    xTf[:, :D, :] = x.transpose(1, 2, 0)
    xTf[:, D, :] = 1.0
    xTf = xTf.reshape(t_steps // xb, xb, NKX, 128, B).transpose(0, 3, 1, 2, 4)
    xTf = np.ascontiguousarray(xTf).astype(bf)

    W_ih = np.asarray(W_ih, np.float32)
    W_hh = np.asarray(W_hh, np.float32)
    bias = (np.asarray(b_ih) + np.asarray(b_hh)).astype(np.float32)
    fc1_w = np.asarray(fc1_w, np.float32)
    fc1_b = np.asarray(fc1_b, np.float32)
    fc2_w = np.asarray(fc2_w, np.float32)
    fc2_b = np.asarray(fc2_b, np.float32)

    in_maps = []
    gate_order = (0, 1, 2, 3)  # i, f, g, o (pytorch row-block order)
    for k in range(N_CORES):
        rows = np.concatenate(
            [np.arange(g * H + k * HSL, g * H + (k + 1) * HSL)
             for g in gate_order])
        wih_sl = W_ih[rows, :]            # [512, 409]
        whh_sl = W_hh[rows, :]            # [512, 1024]
        b_sl = bias[rows]                 # [512]
        wihT_k = np.zeros((NKX * 128, GSL), np.float32)
        wihT_k[:D, :] = wih_sl.T
        wihT_k[D, :] = b_sl
        wihT_k = np.ascontiguousarray(
            wihT_k.reshape(NKX, 128, GSL)).astype(bf)
        whhT_k = np.ascontiguousarray(
            whh_sl.T.reshape(NKH, 128, GSL)).astype(bf)

        cc = k % 4                        # C chunk (cores k and k+4 duplicate)
        crows = np.arange(cc * 128, (cc + 1) * 128)
        fc1wT_k = np.ascontiguousarray(
            fc1_w[crows, :].T.reshape(NKH, 128, 128)).astype(bf)
        fc1b_k = np.ascontiguousarray(fc1_b[crows].reshape(128, 1))
        fc2wT_k = np.ascontiguousarray(0.5 * fc2_w[:, crows].T).astype(bf)
        fc2b_k = np.ascontiguousarray(
            np.broadcast_to(fc2_b[None, :], (128, O))).astype(np.float32)

        in_maps.append({
            "xT": xTf,
            "wihT": wihT_k,
            "whhT": whhT_k,
            "fc1wT": fc1wT_k,
            "fc1b": fc1b_k,
            "fc2wT": fc2wT_k,
            "fc2b": fc2b_k,
        })
    return in_maps


def run(inputs, t_steps=T, trace=False):
    nc = _get_kernel(t_steps)
    in_maps = prep_inputs(
        inputs["m_text"], inputs["m_audio"], inputs["m_video"],
        inputs["W_ih"], inputs["W_hh"], inputs["b_ih"], inputs["b_hh"],
        inputs["fc1_w"], inputs["fc1_b"], inputs["fc2_w"], inputs["fc2_b"],
        t_steps=t_steps)
    res = run_bass_kernel_spmd(
        nc, in_maps, core_ids=list(range(N_CORES)), trace=trace)
    shards = [res.results[k]["out_sh"] for k in range(N_CORES)]
    full = np.concatenate(shards, axis=0)          # [(b t), O] token-major
    out = full.reshape(B, t_steps, O)
    return out, res


def kernel(**inputs) -> np.ndarray:
    t_steps = np.asarray(inputs["m_text"]).shape[1]
    out, _ = run(inputs, t_steps=t_steps)
    return out.astype(np.float32)
